# revision 5
# baseline (speedup 1.0000x reference)
"""Contrastive loss (GRACE-style semi_loss pair) on 8 trn2 NeuronCores.

Math (reference):
    a = z1 / ||z1||_row ; b = z2 / ||z2||_row         (N=8192, D=512)
    refl    = exp(a @ a.T / tau) ; between = exp(a @ b.T / tau)
    l1_i = -log(between_ii / (refl.sum(1) + between.sum(1) - refl_ii))
    l2   = same with (z2, z1) swapped
    loss = mean(0.5 * (l1 + l2))

Key identities used:
  - between2 (for l2) = between.T, so its row sums are COLUMN sums of
    exp(a@b.T/tau) -> one extra cross-core ReduceScatter of [8192] floats,
    no 4th matmul.
  - refl_ii = exp(1/tau) exactly (rows are unit-norm).
  - between_ii needs only dab_i = a_i . b_i (computed row-wise in fp32).
  - l1_i = log(denom1_i) - dab_i/tau ; l2_i = log(denom2_i) - dab_i/tau.

Sharding: data-parallel rows. Every core holds the full (normalized,
bf16-cast, transposed) embeddings as matmul moving operands and its own
1024-row slice as stationary; each core computes its [1024 x 8192] blocks
of the three similarity products S_aa, S_ab, S_bb with fused
exp+row-sum on the ACT engine, column partial sums via ones-matmul on PE,
then ReduceScatter(colsums) + AllReduce(scalar loss).
"""

import numpy as np
from contextlib import ExitStack

import concourse.bass as bass
import concourse.tile as tile
from concourse import bacc, mybir
from concourse.bass_utils import run_bass_kernel_spmd
from concourse.masks import make_identity

N = 8192
D = 512
P = 128
NCORES = 8
LOCAL = N // NCORES            # 1024 rows per core
M_CH = LOCAL // P              # 8 local row chunks of 128
N_CH = N // 512                # 16 column chunks of 512
KC = D // P                    # 4 contraction chunks of 128
TAU = 0.4
EXPD = float(np.exp(1.0 / TAU))   # diagonal of exp(S_aa/tau): rows unit-norm

FP32 = mybir.dt.float32
BF16 = mybir.dt.bfloat16
ALU = mybir.AluOpType
ACTF = mybir.ActivationFunctionType


def _variant_flags():
    import os

    v = os.environ.get("BASS_KERNEL_VARIANT", "")
    return set(f for f in v.split(",") if f)


def _build():
    flags = _variant_flags()
    nc = bacc.Bacc("TRN2", debug=False, num_devices=NCORES)
    z1 = nc.dram_tensor("z1", [N, D], FP32, kind="ExternalInput").ap()
    z2 = nc.dram_tensor("z2", [N, D], FP32, kind="ExternalInput").ap()
    z1l = nc.dram_tensor("z1l", [LOCAL, D], FP32, kind="ExternalInput").ap()
    z2l = nc.dram_tensor("z2l", [LOCAL, D], FP32, kind="ExternalInput").ap()
    loss = nc.dram_tensor("loss", [1, 1], FP32, kind="ExternalOutput").ap()

    with tile.TileContext(nc) as tc, ExitStack() as ctx:
        big = ctx.enter_context(tc.tile_pool(name="big", bufs=1))
        stage = ctx.enter_context(tc.tile_pool(name="stage", bufs=6))
        small = ctx.enter_context(tc.tile_pool(name="small", bufs=1))
        scratch = ctx.enter_context(tc.tile_pool(name="scratch", bufs=2))
        pmm = ctx.enter_context(tc.tile_pool(name="pmm", bufs=4, space="PSUM"))
        pcol = ctx.enter_context(tc.tile_pool(name="pcol", bufs=2, space="PSUM"))
        ptr = ctx.enter_context(tc.tile_pool(name="ptr", bufs=2, space="PSUM"))
        dram = ctx.enter_context(tc.tile_pool(name="dram", bufs=1, space="DRAM"))

        # ---- constants --------------------------------------------------
        ident = small.tile([P, P], BF16, tag="ident", name="ident")
        make_identity(nc, ident)
        ones_bf = small.tile([P, 1], BF16, tag="ones_bf", name="ones_bf")
        nc.vector.memset(ones_bf, 1.0)
        ones_f32 = small.tile([P, 1], FP32, tag="ones_f32", name="ones_f32")
        nc.vector.memset(ones_f32, 1.0)

        # ---- persistent operands ---------------------------------------
        # ATL*[p, k, 128m+q] = normalized local rows, d-major (stationary)
        ATL1 = big.tile([P, KC, LOCAL], BF16, tag="ATL1", name="ATL1")
        ATL2 = big.tile([P, KC, LOCAL], BF16, tag="ATL2", name="ATL2")
        # AT*[n][p, k, f] = normalized all rows 512n+f, d-major (moving)
        AT1 = [
            big.tile([P, KC, 512], BF16, tag=f"AT1_{n}", name=f"AT1_{n}")
            for n in range(N_CH)
        ]
        AT2 = [
            big.tile([P, KC, 512], BF16, tag=f"AT2_{n}", name=f"AT2_{n}")
            for n in range(N_CH)
        ]

        # per-(m) row-sum partials over the 16 column chunks
        rsp_aa = [
            small.tile([P, N_CH], FP32, tag=f"rsp_aa{m}", name=f"rsp_aa{m}")
            for m in range(M_CH)
        ]
        rsp_ab = [
            small.tile([P, N_CH], FP32, tag=f"rsp_ab{m}", name=f"rsp_ab{m}")
            for m in range(M_CH)
        ]
        rsp_bb = [
            small.tile([P, N_CH], FP32, tag=f"rsp_bb{m}", name=f"rsp_bb{m}")
            for m in range(M_CH)
        ]

        ss_l1 = small.tile([P, M_CH], FP32, tag="ss_l1", name="ss_l1")
        ss_l2 = small.tile([P, M_CH], FP32, tag="ss_l2", name="ss_l2")
        u_ab = small.tile([P, M_CH], FP32, tag="u_ab", name="u_ab")
        invn_l1 = small.tile([P, M_CH], FP32, tag="invn_l1", name="invn_l1")
        invn_l2 = small.tile([P, M_CH], FP32, tag="invn_l2", name="invn_l2")
        ss_f1 = small.tile([P, 4 * N_CH], FP32, tag="ss_f1", name="ss_f1")
        ss_f2 = small.tile([P, 4 * N_CH], FP32, tag="ss_f2", name="ss_f2")
        invn_f1 = small.tile([P, 4 * N_CH], FP32, tag="invn_f1", name="invn_f1")
        invn_f2 = small.tile([P, 4 * N_CH], FP32, tag="invn_f2", name="invn_f2")

        # collective buffers
        cc1_in = dram.tile([1, N], FP32, tag="cc1_in", name="cc1_in")
        cc1_out = dram.tile([M_CH, P], FP32, tag="cc1_out", name="cc1_out")
        cc2_in = dram.tile([1, 1], FP32, tag="cc2_in", name="cc2_in")
        cc2_out = dram.tile(
            [1, 1], FP32, tag="cc2_out", name="cc2_out", addr_space="Shared"
        )

        def sumsq(zt, acc_slice, nm):
            # (zt * 1.0) * zt with fused row-sum; TTR is a custom DVE op the
            # terminal runtime rejects, scalar_tensor_tensor is standard.
            sq = scratch.tile([P, D], FP32, tag="sq", name=f"sq_{nm}")
            nc.vector.scalar_tensor_tensor(
                out=sq, in0=zt, scalar=1.0, in1=zt,
                op0=ALU.mult, op1=ALU.mult, accum_out=acc_slice,
            )

        def invn_group(ss_t, invn_t, lo, hi, nm):
            lss = scratch.tile([P, hi - lo], FP32, tag="lss", name=f"lss_{nm}")
            nc.scalar.activation(out=lss, in_=ss_t[:, lo:hi], func=ACTF.Ln)
            nc.scalar.activation(
                out=invn_t[:, lo:hi], in_=lss, func=ACTF.Exp, scale=-0.5
            )

        def scale_transpose(zt, invn_slice, dst, nm, abtag):
            """normalize+cast one [128,512] row tile, transpose to d-major."""
            abf = scratch.tile([P, D], BF16, tag=abtag, name=f"abf_{nm}")
            nc.vector.tensor_scalar_mul(abf, zt, invn_slice)
            pt = ptr.tile([P, KC, P], BF16, tag="tr", name=f"tr_{nm}")
            for k in range(KC):
                nc.tensor.transpose(pt[:, k, :], abf[:, P * k : P * (k + 1)], ident)
            nc.vector.tensor_copy(dst, pt)

        # ---- P1: local rows -> stationary operands + dab ----------------
        for g in range(M_CH // 4):
            zts = []
            for j in range(4):
                t = 4 * g + j
                zt1 = stage.tile([P, D], FP32, tag="st_z1", name=f"zl1_{t}")
                nc.sync.dma_start(out=zt1, in_=z1l[P * t : P * (t + 1), :])
                zt2 = stage.tile([P, D], FP32, tag="st_z2", name=f"zl2_{t}")
                nc.sync.dma_start(out=zt2, in_=z2l[P * t : P * (t + 1), :])
                sumsq(zt1, ss_l1[:, t : t + 1], f"l1_{t}")
                sumsq(zt2, ss_l2[:, t : t + 1], f"l2_{t}")
                sqc = scratch.tile([P, D], FP32, tag="sq", name=f"sq_u_{t}")
                nc.vector.scalar_tensor_tensor(
                    out=sqc, in0=zt1, scalar=1.0, in1=zt2,
                    op0=ALU.mult, op1=ALU.mult, accum_out=u_ab[:, t : t + 1],
                )
                zts.append((zt1, zt2))
            invn_group(ss_l1, invn_l1, 4 * g, 4 * g + 4, f"l1g{g}")
            invn_group(ss_l2, invn_l2, 4 * g, 4 * g + 4, f"l2g{g}")
            for j in range(4):
                t = 4 * g + j
                zt1, zt2 = zts[j]
                scale_transpose(
                    zt1, invn_l1[:, t : t + 1],
                    ATL1[:, :, P * t : P * (t + 1)], f"l1_{t}", "abf1",
                )
                scale_transpose(
                    zt2, invn_l2[:, t : t + 1],
                    ATL2[:, :, P * t : P * (t + 1)], f"l2_{t}", "abf2",
                )

        # dab_i = u_i / (||z1_i|| * ||z2_i||)
        dab = small.tile([P, M_CH], FP32, tag="dab", name="dab")
        nc.vector.tensor_mul(dab, u_ab, invn_l1)
        nc.vector.tensor_mul(dab, dab, invn_l2)

        # ---- P2+P3 interleaved: per column chunk ------------------------
        def p2_group(n, z_ap, ss_t, invn_t, AT_list, sttag, abtag):
            zts = []
            for j in range(4):
                t = 4 * n + j
                zt = stage.tile([P, D], FP32, tag=sttag, name=f"{sttag}_f{t}")
                nc.sync.dma_start(out=zt, in_=z_ap[P * t : P * (t + 1), :])
                sumsq(zt, ss_t[:, t : t + 1], f"{sttag}_f{t}")
                zts.append(zt)
            invn_group(ss_t, invn_t, 4 * n, 4 * n + 4, f"{sttag}_g{n}")
            for j in range(4):
                t = 4 * n + j
                scale_transpose(
                    zts[j], invn_t[:, t : t + 1],
                    AT_list[n][:, :, P * j : P * (j + 1)], f"{sttag}_f{t}", abtag,
                )

        def main_chunk(n):
            colp = pcol.tile([1, 512], FP32, tag="col", name=f"colp_{n}")
            for m in range(M_CH):
                aa = pmm.tile([P, 512], FP32, tag="mm", name=f"aa_{n}_{m}")
                ab = pmm.tile([P, 512], FP32, tag="mm", name=f"ab_{n}_{m}")
                bb = pmm.tile([P, 512], FP32, tag="mm", name=f"bb_{n}_{m}")
                for k in range(KC):
                    nc.tensor.matmul(
                        aa, ATL1[:, k, P * m : P * (m + 1)], AT1[n][:, k, :],
                        start=(k == 0), stop=(k == KC - 1),
                    )
                for k in range(KC):
                    nc.tensor.matmul(
                        ab, ATL1[:, k, P * m : P * (m + 1)], AT2[n][:, k, :],
                        start=(k == 0), stop=(k == KC - 1),
                    )
                for k in range(KC):
                    nc.tensor.matmul(
                        bb, ATL2[:, k, P * m : P * (m + 1)], AT2[n][:, k, :],
                        start=(k == 0), stop=(k == KC - 1),
                    )
                # fused exp + row-sum; aa/bb exp'd in place in PSUM (only the
                # row sums are needed), ab exp'd to SBUF bf16 for the
                # column-sum ones-matmul.
                nc.scalar.activation(
                    out=aa, in_=aa, func=ACTF.Exp, scale=1.0 / TAU,
                    accum_out=rsp_aa[m][:, n : n + 1],
                )
                exab = scratch.tile(
                    [P, 512], BF16, tag="exab", name=f"exab_{n}_{m}", bufs=4
                )
                nc.scalar.activation(
                    out=exab, in_=ab, func=ACTF.Exp, scale=1.0 / TAU,
                    accum_out=rsp_ab[m][:, n : n + 1],
                )
                nc.tensor.matmul(
                    colp, ones_bf, exab, start=(m == 0), stop=(m == M_CH - 1)
                )
                nc.scalar.activation(
                    out=bb, in_=bb, func=ACTF.Exp, scale=1.0 / TAU,
                    accum_out=rsp_bb[m][:, n : n + 1],
                )
            csb = scratch.tile([1, 512], FP32, tag="csb", name=f"csb_{n}")
            nc.vector.tensor_copy(csb, colp)
            nc.sync.dma_start(out=cc1_in[:, 512 * n : 512 * (n + 1)], in_=csb)

        for n in range(N_CH):
            p2_group(n, z1, ss_f1, invn_f1, AT1, "st_z1", "abf1")
            p2_group(n, z2, ss_f2, invn_f2, AT2, "st_z2", "abf2")
            main_chunk(n)

        # ---- P4: epilogue ----------------------------------------------
        rs_aa = small.tile([P, M_CH], FP32, tag="rs_aa", name="rs_aa")
        rs_ab = small.tile([P, M_CH], FP32, tag="rs_ab", name="rs_ab")
        rs_bb = small.tile([P, M_CH], FP32, tag="rs_bb", name="rs_bb")
        for m in range(M_CH):
            nc.vector.reduce_sum(
                out=rs_aa[:, m : m + 1], in_=rsp_aa[m], axis=mybir.AxisListType.X
            )
            nc.vector.reduce_sum(
                out=rs_ab[:, m : m + 1], in_=rsp_ab[m], axis=mybir.AxisListType.X
            )
            nc.vector.reduce_sum(
                out=rs_bb[:, m : m + 1], in_=rsp_bb[m], axis=mybir.AxisListType.X
            )

        # denom1 = rowsum_aa + rowsum_ab - exp(1/tau)
        denom1 = small.tile([P, M_CH], FP32, tag="denom1", name="denom1")
        nc.vector.scalar_tensor_tensor(
            out=denom1, in0=rs_aa, scalar=-EXPD, in1=rs_ab,
            op0=ALU.add, op1=ALU.add,
        )

        # colsum partials -> ReduceScatter -> this core's 1024 columns
        nc.gpsimd.collective_compute(
            "ReduceScatter",
            ALU.add,
            replica_groups=[list(range(NCORES))],
            ins=[cc1_in.opt()],
            outs=[cc1_out.opt()],
        )
        colsum_l = small.tile([P, M_CH], FP32, tag="colsum_l", name="colsum_l")
        nc.sync.dma_start(out=colsum_l, in_=cc1_out.rearrange("m p -> p m"))

        denom2 = small.tile([P, M_CH], FP32, tag="denom2", name="denom2")
        nc.vector.scalar_tensor_tensor(
            out=denom2, in0=rs_bb, scalar=-EXPD, in1=colsum_l,
            op0=ALU.add, op1=ALU.add,
        )

        nc.scalar.activation(out=denom1, in_=denom1, func=ACTF.Ln)
        nc.scalar.activation(out=denom2, in_=denom2, func=ACTF.Ln)
        nc.vector.tensor_add(denom1, denom1, denom2)  # ld1 + ld2

        combo = scratch.tile([P, M_CH], FP32, tag="combo", name="combo")
        ppart = small.tile([P, 1], FP32, tag="ppart", name="ppart")
        nc.vector.scalar_tensor_tensor(
            out=combo, in0=dab, scalar=-2.0 / TAU, in1=denom1,
            op0=ALU.mult, op1=ALU.add, accum_out=ppart,
        )
        lps = pcol.tile([1, 1], FP32, tag="col", name="lps")
        nc.tensor.matmul(lps, ones_f32, ppart, start=True, stop=True)
        lsb = small.tile([1, 1], FP32, tag="lsb", name="lsb")
        nc.scalar.mul(lsb, lps, 0.5 / N)

        nc.sync.dma_start(out=cc2_in, in_=lsb)
        nc.gpsimd.collective_compute(
            "AllReduce",
            ALU.add,
            replica_groups=[list(range(NCORES))],
            ins=[cc2_in.opt()],
            outs=[cc2_out.opt()],
        )
        nc.sync.dma_start(out=loss, in_=cc2_out)

    nc.compile()
    return nc


_NC_CACHE = None


def _get_nc():
    global _NC_CACHE
    if _NC_CACHE is None:
        _NC_CACHE = _build()
    return _NC_CACHE


def _in_maps(z1, z2):
    z1 = np.ascontiguousarray(np.asarray(z1), dtype=np.float32)
    z2 = np.ascontiguousarray(np.asarray(z2), dtype=np.float32)
    maps = []
    for c in range(NCORES):
        sl = slice(LOCAL * c, LOCAL * (c + 1))
        maps.append(
            {
                "z1": z1,
                "z2": z2,
                "z1l": np.ascontiguousarray(z1[sl]),
                "z2l": np.ascontiguousarray(z2[sl]),
            }
        )
    return maps


def kernel(z1, z2):
    nc = _get_nc()
    res = run_bass_kernel_spmd(nc, _in_maps(z1, z2), list(range(NCORES)))
    return np.asarray(res.results[0]["loss"], dtype=np.float32).reshape(())


def kernel_traced(z1, z2):
    """Same as kernel() but with NTFF profiling; returns (loss, exec_time_ns,
    trace_path)."""
    import concourse.bass_utils as bu

    bu.upload_artifacts = lambda tmpdir: "local://" + tmpdir  # no egress
    nc = _get_nc()
    res = run_bass_kernel_spmd(
        nc, _in_maps(z1, z2), list(range(NCORES)), trace=True
    )
    out = np.asarray(res.results[0]["loss"], dtype=np.float32).reshape(())
    trace_path = (
        res.instructions_and_trace[1] if res.instructions_and_trace else None
    )
    return out, res.exec_time_ns, trace_path
